# revision 48
# baseline (speedup 1.0000x reference)
"""Multi-head attention (B=2, S=2048, H=1024, 16 heads x 64d) on 8 trn2 cores.

Sharding: tensor-parallel over heads (2 heads/core). Each core computes the
qkv projection for its 384 output features, attention for its 2 heads, and a
partial o_proj ([4096,1024] over its 128-feature slice). Host sums the 8
partials and adds b_o plus the v-bias fold (w_o @ bv — the v bias shifts the
attention output by a constant row, so it moves to the host for free).

Device layout (per core, feature-major):
  QT/KT [128, 4096]  rows = head_local*64 + d, cols = b*2048 + s  (fp16)
  The k bias is dropped entirely (adds a per-query constant to scores, which
  softmax cancels); the q bias folds into the ACT-engine PSUM->SBUF copy.
  V is copied to fp16 feature-major then moved to key-major Vaug slabs by
  XBAR transpose DMAs (no PE transposes). S^T orientation for scores ([k,q])
  so the softmax sum falls out of the PV matmul via a ones-column in Vaug;
  exp runs on ScalarE from PSUM; normalization = reciprocal of the sums row
  + ones-broadcast matmul + PSUMxPSUM multiply (no staging copy).
Matmuls run in fp16 (measured end-to-end rel err ~8e-4); the softmax
normalizer chain stays fp32r. Emission keeps PE saturated: a minimal head
(b0 k+v+q0) starts attention early, and a fine-grained filler queue (rest of
qkv / o_proj token tiles) drains inside exp shadows.
"""
import sys

sys.path.insert(0, "/opt/trn_rl_repo")
import numpy as np

NHEADS = 16
HEAD_DIM = 64
HIDDEN = 1024
QKV = NHEADS * HEAD_DIM  # 1024
SCALING = HEAD_DIM ** -0.5
B = 2
S = 2048
T = B * S  # 4096
NCORES = 8
HPC = NHEADS // NCORES  # 2 heads per core
FEAT = HPC * HEAD_DIM  # 128
CHUNK = 512
NCHUNK = S // CHUNK  # 4 per batch
KSLABS = HIDDEN // 128  # 8
SSLABS = S // 128  # 16
D1 = HEAD_DIM + 1  # 65

_CACHE = {}
LAST_RESULT = None  # BassKernelResults of the most recent kernel() call


def _split_waits(nc, keep=1):
    """Hoist excess per-instruction sem waits into standalone EventSemaphore
    instructions (walrus codegen has small per-opcode wait budgets)."""
    import bass_rust
    import concourse.mybir as mybir

    n_new = 0
    for f in nc.m.functions:
        for blk in f.blocks:
            out = []
            changed = False
            for inst in blk.instructions:
                si = inst.sync_info
                waits = list(si.on_wait) if si is not None else []
                if len(waits) > keep:
                    excess = waits[: len(waits) - keep]
                    kept = waits[len(waits) - keep:]
                    for w in excess:
                        out.append(mybir.InstEventSemaphore(
                            name=f"{inst.name}-esw{n_new}",
                            engine=inst.engine,
                            sync_info=bass_rust.SyncInfo(on_wait=[w], on_update=[]),
                        ))
                        n_new += 1
                    inst.sync_info = bass_rust.SyncInfo(
                        on_wait=kept, on_update=list(si.on_update))
                    changed = True
                out.append(inst)
            if changed:
                blk.instructions = out
    return n_new


def _build(reps=1, use_xbar=False, use_act_bias=True, slab_dma=True):
    import concourse.bass as bass
    import concourse.mybir as mybir
    import concourse.tile as tile
    from concourse.masks import make_identity

    f32 = mybir.dt.float32
    f32r = mybir.dt.float32r
    f16 = mybir.dt.float16
    Exp = mybir.ActivationFunctionType.Exp
    Ident = mybir.ActivationFunctionType.Identity

    nc = bass.Bass()
    xT = nc.dram_tensor("xT", [HIDDEN, T], f16, kind="ExternalInput")
    # p-major weight layout: [m, partition, slab, feat] so each partition's
    # slice is one contiguous 2KB DMA descriptor
    wq3 = nc.dram_tensor("wq3", [3, 128, KSLABS, FEAT], f16,
                         kind="ExternalInput")
    bq = nc.dram_tensor("bq", [FEAT, 1], f32, kind="ExternalInput")
    woT = nc.dram_tensor("woT", [FEAT, HIDDEN], f16, kind="ExternalInput")
    out_d = nc.dram_tensor("out", [T, HIDDEN], f16, kind="ExternalOutput")

    with tile.TileContext(nc) as tc, nc.allow_low_precision(reason="fp16 matmuls"):
        with (
            tc.tile_pool(name="sing", bufs=1) as sing,
            tc.tile_pool(name="xp", bufs=9) as xp,
            tc.tile_pool(name="pp", bufs=3) as pp,
            tc.tile_pool(name="stg", bufs=1) as stg,
            tc.tile_pool(name="sm", bufs=2) as sm,
            tc.tile_pool(name="op", bufs=3) as op,
            tc.tile_pool(name="ps_s", bufs=2, space="PSUM") as ps_s,
            tc.tile_pool(name="ps_o", bufs=2, space="PSUM") as ps_o,
            tc.tile_pool(name="ps_mm", bufs=2, space="PSUM") as ps_mm,
        ):
            wq_sb = [sing.tile([128, KSLABS, FEAT], f16, tag=f"wq{m}",
                               name=f"wq{m}") for m in range(3)]
            wo_sb = sing.tile([FEAT, HIDDEN], f16, tag="wo")
            bq_sb = sing.tile([FEAT, 1], f32, tag="bq")
            ones1 = sing.tile([1, HEAD_DIM], f32r, tag="on")
            QT = sing.tile([128, T], f16, tag="qt")
            KT = sing.tile([128, T], f16, tag="kt")
            VT = sing.tile([128, T], f16, tag="vt")
            OT = sing.tile([128, T], f16, tag="ot")
            Vaug = sing.tile([128, B, SSLABS, HPC, D1], f16, tag="va")

            xT_c = xT[:].rearrange("(s p) t -> p s t", p=128)

            # DMA order tracks first-use: k weights + x chunk 0 (per-slab)
            # gate the first matmuls, then v/q weights + bias, then the
            # remaining b0 chunks, then wo
            nc.sync.dma_start(out=wq_sb[1], in_=wq3[1])
            xc0 = xp.tile([128, KSLABS, CHUNK], f16, tag="xc", name="xc0")
            if slab_dma:
                for s in range(KSLABS):
                    nc.sync.dma_start(out=xc0[:, s, :], in_=xT_c[:, s, 0:CHUNK])
            else:
                nc.sync.dma_start(out=xc0, in_=xT_c[:, :, 0:CHUNK])
            nc.sync.dma_start(out=wq_sb[2], in_=wq3[2])
            nc.sync.dma_start(out=wq_sb[0], in_=wq3[0])
            nc.sync.dma_start(out=bq_sb, in_=bq[:])
            ones_f = stg.tile([1, HEAD_DIM], f32, tag="onf")
            nc.vector.memset(ones_f, 1.0)
            nc.vector.tensor_copy(ones1, ones_f)
            nc.vector.memset(Vaug[:, :, :, :, HEAD_DIM:D1], 1.0)
            ident = None
            if not use_xbar:
                ident = sing.tile([128, 128], f16, tag="id")
                make_identity(nc, ident)

            from collections import deque
            filler = deque()  # entries: (pe_weight, thunk)

            def drain(target=1, cap=6):
                # pop until ~target units (~427ns each) of PE work emitted;
                # zero-weight thunks (copies/allocs/flushes) don't count
                acc = 0
                pops = 0
                while filler and acc < target and pops < cap:
                    w, fn = filler.popleft()
                    fn()
                    acc += w
                    pops += 1

            def drain_all():
                while filler:
                    filler.popleft()[1]()

            def qkv_dma(b, n):
                g = b * NCHUNK + n
                xc = xp.tile([128, KSLABS, CHUNK], f16, tag="xc", name="xc")
                nc.sync.dma_start(out=xc, in_=xT_c[:, :, g * CHUNK:(g + 1) * CHUNK])
                return xc

            def qkv_mm(b, n, m, xc, box, s0, s1):
                # two accumulating slab-matmuls of one q/k/v chunk
                if s0 == 0:
                    box["acc"] = ps_mm.tile([128, CHUNK], f32, tag="mm",
                                            name="acc")
                for s in range(s0, s1):
                    nc.tensor.matmul(
                        box["acc"], wq_sb[m][:, s, :], xc[:, s, :],
                        start=(s == 0), stop=(s == KSLABS - 1))

            def qkv_fin(b, n, m, box):
                # PSUM->SBUF copy (ACT for q with bias fold, DVE else)
                g = b * NCHUNK + n
                lo, hi = g * CHUNK, (g + 1) * CHUNK
                acc = box["acc"]
                if m == 0:
                    if use_act_bias:
                        nc.scalar.activation(
                            out=QT[:, lo:hi], in_=acc, func=Ident, bias=bq_sb)
                    else:
                        nc.vector.tensor_scalar_add(QT[:, lo:hi], acc, bq_sb)
                elif m == 1:
                    nc.vector.tensor_copy(KT[:, lo:hi], acc)
                else:
                    nc.vector.tensor_copy(VT[:, lo:hi], acc)
                    if use_xbar:
                        for h in range(HPC):
                            nc.sync.dma_start_transpose(
                                Vaug[:, b, 4 * n:4 * n + 4, h, 0:HEAD_DIM],
                                VT[64 * h:64 * h + 64, lo:hi])
                    else:
                        for h in range(HPC):
                            for k in range(4 * n, 4 * n + 4):
                                tp = ps_mm.tile([128, HEAD_DIM], f16,
                                                tag="mm", name="tp")
                                nc.tensor.transpose(
                                    tp,
                                    VT[64 * h:64 * h + 64,
                                       b * S + 128 * k:b * S + 128 * (k + 1)],
                                    ident[64 * h:64 * h + 64,
                                          64 * h:64 * h + 64])
                                nc.vector.tensor_copy(
                                    Vaug[:, b, k, h, 0:HEAD_DIM], tp)

            def qkv_feat(b, n, m, xc):
                box = {}
                for s0 in range(0, KSLABS, 2):
                    qkv_mm(b, n, m, xc, box, s0, s0 + 2)
                qkv_fin(b, n, m, box)

            def qkv_thunks(b, n, m, xc):
                box = {}
                th = [(1, lambda s0=s0: qkv_mm(b, n, m, xc, box, s0, s0 + 2))
                      for s0 in range(0, KSLABS, 2)]
                th.append((0, lambda: qkv_fin(b, n, m, box)))
                return th

            GRP = 2  # S-slabs per exp group (ps_s holds GRP banks x 2 bufs)

            def attn_unit(b, h, qc, dr=None, rb_act=False):
                qlo = b * S + qc * CHUNK
                qsl = slice(qlo, qlo + CHUNK)
                o_ps = ps_o.tile([D1, CHUNK], f32, tag="o")
                ngrp = SSLABS // GRP

                def adr():
                    if dr is not None:
                        return dr
                    # mild draining: one PE-unit per exp shadow, two only
                    # when the backlog is deep (tuned by sim scan)
                    return 2 if len(filler) > 80 else 1

                def s_group(grp):
                    s_ps = ps_s.tile([128, GRP, CHUNK], f32, tag="s", name="s_ps")
                    for kk in range(GRP):
                        k = grp * GRP + kk
                        nc.tensor.matmul(
                            s_ps[:, kk, :],
                            KT[64 * h:64 * h + 64,
                               b * S + 128 * k: b * S + 128 * (k + 1)],
                            QT[64 * h:64 * h + 64, qsl],
                            start=True, stop=True)
                    pt = pp.tile([128, GRP, CHUNK], f16, tag="pt", name="pt")
                    nc.scalar.activation(out=pt, in_=s_ps, func=Exp)
                    return pt

                def pv_group(grp, pt):
                    for kk in range(GRP):
                        k = grp * GRP + kk
                        nc.tensor.matmul(
                            o_ps, Vaug[:, b, k, h, :], pt[:, kk, :],
                            start=(k == 0), stop=(k == SSLABS - 1))

                prev = s_group(0)
                for grp in range(1, ngrp):
                    cur = s_group(grp)
                    drain(adr())  # filler PE work runs in exp(grp-1)'s shadow
                    pv_group(grp - 1, prev)
                    prev = cur
                drain(adr())
                pv_group(ngrp - 1, prev)
                rec = sm.tile([1, CHUNK], f32r, tag="rec")
                nc.vector.reciprocal(rec, o_ps[HEAD_DIM:D1, :])
                b_ps = ps_mm.tile([HEAD_DIM, CHUNK], f32, tag="mm", name="bps")
                nc.tensor.matmul(b_ps, ones1, rec, start=True, stop=True)
                rb = sm.tile([HEAD_DIM, CHUNK], f32, tag="rb")
                # DVE can read only one PSUM operand: stage rb in SBUF (on
                # ACT for the tail units, where DVE is on the critical path)
                if rb_act:
                    nc.scalar.copy(rb, b_ps)
                else:
                    nc.vector.tensor_copy(rb, b_ps)
                nc.vector.tensor_mul(
                    OT[64 * h:64 * h + 64, qsl], o_ps[0:HEAD_DIM, :], rb)

            def oproj_tile_thunks(t, eng="dve"):
                # token tile t (tokens 128t..128t+127): 2 matmul halves into
                # PSUM, copy to an fp16 staging tile, one out-DMA
                box = {}

                def alloc():
                    box["ost"] = op.tile([128, HIDDEN], f16, tag="ost", name="ost")

                def half(nh):
                    acc = ps_mm.tile([128, CHUNK], f32, tag="mm", name="acc2")
                    nc.tensor.matmul(
                        acc, OT[:, 128 * t:128 * (t + 1)],
                        wo_sb[:, nh * CHUNK:(nh + 1) * CHUNK],
                        start=True, stop=True)
                    cp = nc.scalar.copy if eng == "act" else nc.vector.tensor_copy
                    cp(box["ost"][:, nh * CHUNK:(nh + 1) * CHUNK], acc)

                def flush():
                    nc.sync.dma_start(
                        out=out_d[128 * t:128 * (t + 1), :], in_=box["ost"])

                return [(0, alloc), (1, lambda: half(0)),
                        (1, lambda: half(1)), (0, flush)]

            # ---- emission: software-pipelined across reps. Rep r's b1
            # attention phase drains rep r+1's b0 qkv as filler (disjoint
            # QT/KT/VT/Vaug regions, so only WAR deps on finished readers),
            # so the reps-differenced steady state has no head/tail bubble.
            def queue_b0_qkv(xcs, head_too):
                if head_too:
                    for m in (1, 2, 0):
                        filler.extend(qkv_thunks(0, 0, m, xcs[0, 0]))
                for n in range(1, NCHUNK):
                    filler.append((4, lambda n=n: qkv_feat(0, n, 1, xcs[0, n])))
                    filler.append((4, lambda n=n: qkv_feat(0, n, 2, xcs[0, n])))
                for n in range(1, NCHUNK):
                    filler.extend(qkv_thunks(0, n, 0, xcs[0, n]))

            def queue_b1_qkv(xcs):
                for n in range(NCHUNK):
                    for m in (1, 2, 0):
                        filler.extend(qkv_thunks(1, n, m, xcs[1, n]))

            xcs_cur = {(0, 0): xc0}
            for n in range(1, NCHUNK):
                xcs_cur[0, n] = qkv_dma(0, n)
            for n in range(NCHUNK):
                xcs_cur[1, n] = qkv_dma(1, n)

            # rep 0 head emitted directly (nothing to overlap with yet)
            qkv_feat(0, 0, 1, xcs_cur[0, 0])
            qkv_feat(0, 0, 2, xcs_cur[0, 0])
            qkv_feat(0, 0, 0, xcs_cur[0, 0])
            nc.sync.dma_start(out=wo_sb, in_=woT[:])
            queue_b0_qkv(xcs_cur, head_too=False)
            queue_b1_qkv(xcs_cur)

            for _rep in range(reps):
                final = _rep == reps - 1
                for qc in range(NCHUNK):
                    first = _rep == 0 and qc == 0
                    attn_unit(0, 0, qc, dr=1 if first else None)
                    attn_unit(0, 1, qc, dr=1 if first else None)
                    for t in range(4 * qc, 4 * qc + 4):
                        filler.extend(oproj_tile_thunks(t))
                if not final:
                    xcs_nxt = {}
                    for n in range(NCHUNK):
                        xcs_nxt[0, n] = qkv_dma(0, n)
                for qc in range(NCHUNK):
                    last = final and qc == NCHUNK - 1
                    attn_unit(1, 0, qc, dr=1 if last else None,
                              rb_act=last)
                    attn_unit(1, 1, qc, dr=1 if last else None,
                              rb_act=last)
                    eng = "act" if last else "dve"
                    for t in range(16 + 4 * qc, 16 + 4 * qc + 4):
                        filler.extend(oproj_tile_thunks(t, eng))
                    if not final and qc == 0:
                        queue_b0_qkv(xcs_nxt, head_too=True)
                if not final:
                    for n in range(NCHUNK):
                        xcs_nxt[1, n] = qkv_dma(1, n)
                    queue_b1_qkv(xcs_nxt)
                    xcs_cur = xcs_nxt
            drain_all()

    _split_waits(nc)
    return nc


def make_in_maps(hidden_states, w_qkv, b_qkv, w_o, b_o):
    x16 = np.ascontiguousarray(
        np.asarray(hidden_states, dtype=np.float32).reshape(T, HIDDEN).T
    ).astype(np.float16)
    w_qkv = np.asarray(w_qkv, dtype=np.float32)
    b_qkv = np.asarray(b_qkv, dtype=np.float32)
    w_o = np.asarray(w_o, dtype=np.float32)

    in_maps = []
    for c in range(NCORES):
        rq = slice(c * FEAT, (c + 1) * FEAT)
        wq = w_qkv[0:QKV][rq] * SCALING
        wk = w_qkv[QKV:2 * QKV][rq]
        wv = w_qkv[2 * QKV:3 * QKV][rq]
        bqc = b_qkv[0:QKV][rq] * SCALING
        wqkvT = np.concatenate([wq, wk, wv], axis=0).T  # [1024, 384]
        wq3 = np.ascontiguousarray(
            wqkvT.reshape(KSLABS, 128, 3, FEAT).transpose(2, 1, 0, 3)
        ).astype(np.float16)
        in_maps.append({
            "xT": x16,
            "wq3": wq3,
            "bq": np.ascontiguousarray(bqc[:, None]),
            "woT": np.ascontiguousarray(w_o[:, rq].T).astype(np.float16),
        })
    return in_maps


def kernel(hidden_states, w_qkv, b_qkv, w_o, b_o):
    global LAST_RESULT
    from concourse.bass_utils import run_bass_kernel_spmd
    import os

    variant = os.environ.get("KERNEL_VARIANT", "")
    kw = dict(
        use_xbar="xbar" in variant and "noxbar" not in variant,
        use_act_bias="noactbias" not in variant,
        slab_dma="noslab" not in variant,
    )
    key = ("nc", variant)
    if key not in _CACHE:
        _CACHE[key] = _build(**kw)
    nc = _CACHE[key]

    b_qkv = np.asarray(b_qkv, dtype=np.float32)
    w_o = np.asarray(w_o, dtype=np.float32)
    b_o = np.asarray(b_o, dtype=np.float32)
    in_maps = make_in_maps(hidden_states, w_qkv, b_qkv, w_o, b_o)

    trace = bool(os.environ.get("KERNEL_TRACE"))
    res = run_bass_kernel_spmd(nc, in_maps, list(range(NCORES)), trace=trace)
    LAST_RESULT = res

    acc = np.zeros((T, HIDDEN), dtype=np.float64)
    for c in range(NCORES):
        acc += res.results[c]["out"]
    bv = b_qkv[2 * QKV:3 * QKV]
    acc += (b_o + w_o @ bv)
    out = acc.astype(np.float32).reshape(B, S, HIDDEN)
    return out


# revision 53
# speedup vs baseline: 1.0947x; 1.0947x over previous
"""Multi-head attention (B=2, S=2048, H=1024, 16 heads x 64d) on 8 trn2 cores.

Sharding: tensor-parallel over heads (2 heads/core). Each core computes the
qkv projection for its 384 output features, attention for its 2 heads, and a
partial o_proj ([4096,1024] over its 128-feature slice). Host sums the 8
partials and adds b_o plus the v-bias fold (w_o @ bv — the v bias shifts the
attention output by a constant row, so it moves to the host for free).

Device layout (per core, feature-major):
  QT/KT [128, 4096]  rows = head_local*64 + d, cols = b*2048 + s  (fp16)
  The k bias is dropped entirely (adds a per-query constant to scores, which
  softmax cancels); the q bias folds into the ACT-engine PSUM->SBUF copy.
  V is copied to fp16 feature-major then moved to key-major Vaug slabs by
  XBAR transpose DMAs (no PE transposes). S^T orientation for scores ([k,q])
  so the softmax sum falls out of the PV matmul via a ones-column in Vaug;
  exp runs on ScalarE from PSUM; normalization = reciprocal of the sums row
  + ones-broadcast matmul + PSUMxPSUM multiply (no staging copy).
Matmuls run in fp16 (measured end-to-end rel err ~8e-4); the softmax
normalizer chain stays fp32r. Emission keeps PE saturated: a minimal head
(b0 k+v+q0) starts attention early, and a fine-grained filler queue (rest of
qkv / o_proj token tiles) drains inside exp shadows.
"""
import sys

sys.path.insert(0, "/opt/trn_rl_repo")
import numpy as np

NHEADS = 16
HEAD_DIM = 64
HIDDEN = 1024
QKV = NHEADS * HEAD_DIM  # 1024
SCALING = HEAD_DIM ** -0.5
B = 2
S = 2048
T = B * S  # 4096
NCORES = 8
HPC = NHEADS // NCORES  # 2 heads per core
FEAT = HPC * HEAD_DIM  # 128
CHUNK = 512
NCHUNK = S // CHUNK  # 4 per batch
KSLABS = HIDDEN // 128  # 8
SSLABS = S // 128  # 16
D1 = HEAD_DIM + 1  # 65

_CACHE = {}
LAST_RESULT = None  # BassKernelResults of the most recent kernel() call


def _split_waits(nc, keep=1):
    """Hoist excess per-instruction sem waits into standalone EventSemaphore
    instructions (walrus codegen has small per-opcode wait budgets)."""
    import bass_rust
    import concourse.mybir as mybir

    n_new = 0
    for f in nc.m.functions:
        for blk in f.blocks:
            out = []
            changed = False
            for inst in blk.instructions:
                si = inst.sync_info
                waits = list(si.on_wait) if si is not None else []
                if len(waits) > keep:
                    excess = waits[: len(waits) - keep]
                    kept = waits[len(waits) - keep:]
                    for w in excess:
                        out.append(mybir.InstEventSemaphore(
                            name=f"{inst.name}-esw{n_new}",
                            engine=inst.engine,
                            sync_info=bass_rust.SyncInfo(on_wait=[w], on_update=[]),
                        ))
                        n_new += 1
                    inst.sync_info = bass_rust.SyncInfo(
                        on_wait=kept, on_update=list(si.on_update))
                    changed = True
                out.append(inst)
            if changed:
                blk.instructions = out
    return n_new


def _build(reps=1, use_xbar=False, use_act_bias=True, slab_dma=True):
    import concourse.bass as bass
    import concourse.mybir as mybir
    import concourse.tile as tile
    from concourse.masks import make_identity

    f32 = mybir.dt.float32
    f32r = mybir.dt.float32r
    f16 = mybir.dt.float16
    Exp = mybir.ActivationFunctionType.Exp
    Ident = mybir.ActivationFunctionType.Identity

    nc = bass.Bass()
    xT = nc.dram_tensor("xT", [HIDDEN, T], f16, kind="ExternalInput")
    # p-major weight layout: [m, partition, slab, feat] so each partition's
    # slice is one contiguous 2KB DMA descriptor
    wq3 = nc.dram_tensor("wq3", [3, 128, KSLABS, FEAT], f16,
                         kind="ExternalInput")
    bq = nc.dram_tensor("bq", [FEAT, 1], f32, kind="ExternalInput")
    woT = nc.dram_tensor("woT", [FEAT, HIDDEN], f16, kind="ExternalInput")
    out_d = nc.dram_tensor("out", [T, HIDDEN], f16, kind="ExternalOutput")

    with tile.TileContext(nc) as tc, nc.allow_low_precision(reason="fp16 matmuls"):
        with (
            tc.tile_pool(name="sing", bufs=1) as sing,
            tc.tile_pool(name="xp", bufs=9) as xp,
            tc.tile_pool(name="pp", bufs=6) as pp,
            tc.tile_pool(name="stg", bufs=1) as stg,
            tc.tile_pool(name="sm", bufs=2) as sm,
            tc.tile_pool(name="op", bufs=3) as op,
            tc.tile_pool(name="ps_s", bufs=2, space="PSUM") as ps_s,
            tc.tile_pool(name="ps_o", bufs=2, space="PSUM") as ps_o,
            tc.tile_pool(name="ps_mm", bufs=2, space="PSUM") as ps_mm,
        ):
            wq_sb = [sing.tile([128, KSLABS, FEAT], f16, tag=f"wq{m}",
                               name=f"wq{m}") for m in range(3)]
            wo_sb = sing.tile([FEAT, HIDDEN], f16, tag="wo")
            bq_sb = sing.tile([FEAT, 1], f32, tag="bq")
            ones1 = sing.tile([1, HEAD_DIM], f32r, tag="on")
            QT = sing.tile([128, T], f16, tag="qt")
            KT = sing.tile([128, T], f16, tag="kt")
            VT = sing.tile([128, T], f16, tag="vt")
            OT = sing.tile([128, T], f16, tag="ot")
            Vaug = sing.tile([128, B, SSLABS, HPC, D1], f16, tag="va")

            xT_c = xT[:].rearrange("(s p) t -> p s t", p=128)

            # DMA order tracks first-use: k weights + x chunk 0 (per-slab)
            # gate the first matmuls, then v/q weights + bias, then the
            # remaining b0 chunks, then wo
            nc.sync.dma_start(out=wq_sb[1], in_=wq3[1])
            xc0 = xp.tile([128, KSLABS, CHUNK], f16, tag="xc", name="xc0")
            if slab_dma:
                for s in range(KSLABS):
                    nc.sync.dma_start(out=xc0[:, s, :], in_=xT_c[:, s, 0:CHUNK])
            else:
                nc.sync.dma_start(out=xc0, in_=xT_c[:, :, 0:CHUNK])
            nc.sync.dma_start(out=wq_sb[2], in_=wq3[2])
            nc.sync.dma_start(out=wq_sb[0], in_=wq3[0])
            nc.sync.dma_start(out=bq_sb, in_=bq[:])
            ones_f = stg.tile([1, HEAD_DIM], f32, tag="onf")
            nc.vector.memset(ones_f, 1.0)
            nc.vector.tensor_copy(ones1, ones_f)
            nc.vector.memset(Vaug[:, :, :, :, HEAD_DIM:D1], 1.0)
            ident = None
            if not use_xbar:
                ident = sing.tile([128, 128], f16, tag="id")
                make_identity(nc, ident)

            from collections import deque
            filler = deque()  # entries: (pe_weight, thunk)

            def drain(target=1, cap=6):
                # pop until ~target units (~427ns each) of PE work emitted;
                # zero-weight thunks (copies/allocs/flushes) don't count
                acc = 0
                pops = 0
                while filler and acc < target and pops < cap:
                    w, fn = filler.popleft()
                    fn()
                    acc += w
                    pops += 1

            def drain_all():
                while filler:
                    filler.popleft()[1]()

            def qkv_dma(b, n):
                g = b * NCHUNK + n
                xc = xp.tile([128, KSLABS, CHUNK], f16, tag="xc", name="xc")
                nc.sync.dma_start(out=xc, in_=xT_c[:, :, g * CHUNK:(g + 1) * CHUNK])
                return xc

            def qkv_mm(b, n, m, xc, box, s0, s1):
                # two accumulating slab-matmuls of one q/k/v chunk
                if s0 == 0:
                    box["acc"] = ps_mm.tile([128, CHUNK], f32, tag="mm",
                                            name="acc")
                for s in range(s0, s1):
                    nc.tensor.matmul(
                        box["acc"], wq_sb[m][:, s, :], xc[:, s, :],
                        start=(s == 0), stop=(s == KSLABS - 1))

            def qkv_fin(b, n, m, box):
                # PSUM->SBUF copy (ACT for q with bias fold, DVE else)
                g = b * NCHUNK + n
                lo, hi = g * CHUNK, (g + 1) * CHUNK
                acc = box["acc"]
                if m == 0:
                    if use_act_bias:
                        nc.scalar.activation(
                            out=QT[:, lo:hi], in_=acc, func=Ident, bias=bq_sb)
                    else:
                        nc.vector.tensor_scalar_add(QT[:, lo:hi], acc, bq_sb)
                elif m == 1:
                    nc.vector.tensor_copy(KT[:, lo:hi], acc)
                else:
                    nc.vector.tensor_copy(VT[:, lo:hi], acc)
                    if use_xbar:
                        for h in range(HPC):
                            nc.sync.dma_start_transpose(
                                Vaug[:, b, 4 * n:4 * n + 4, h, 0:HEAD_DIM],
                                VT[64 * h:64 * h + 64, lo:hi])
                    else:
                        for h in range(HPC):
                            for k in range(4 * n, 4 * n + 4):
                                tp = ps_mm.tile([128, HEAD_DIM], f16,
                                                tag="mm", name="tp")
                                nc.tensor.transpose(
                                    tp,
                                    VT[64 * h:64 * h + 64,
                                       b * S + 128 * k:b * S + 128 * (k + 1)],
                                    ident[64 * h:64 * h + 64,
                                          64 * h:64 * h + 64])
                                nc.vector.tensor_copy(
                                    Vaug[:, b, k, h, 0:HEAD_DIM], tp)

            def qkv_feat(b, n, m, xc):
                box = {}
                for s0 in range(0, KSLABS, 2):
                    qkv_mm(b, n, m, xc, box, s0, s0 + 2)
                qkv_fin(b, n, m, box)

            def qkv_thunks(b, n, m, xc):
                box = {}
                th = [(1, lambda s0=s0: qkv_mm(b, n, m, xc, box, s0, s0 + 2))
                      for s0 in range(0, KSLABS, 2)]
                th.append((0, lambda: qkv_fin(b, n, m, box)))
                return th

            GRP = 2  # S-slabs per exp group (ps_s holds GRP banks x 2 bufs)
            # PV runs PVLAG groups behind exp: PE reads pt tiles the ACT
            # engine wrote ~2-3 groups earlier. Measured on HW (probe2
            # pv-mode): matmuls consuming freshly-cross-engine-written SBUF
            # data run ~2x slow; ~4-6us of temporal separation restores
            # full rate (2440 -> 1218 ns/group).
            PVLAG = 3
            attq = []  # deferred pv/norm/oproj-queue closures, cross-unit

            def attq_run():
                attq.pop(0)()

            def attn_unit(b, h, qc, dr=None, rb_act=False):
                qlo = b * S + qc * CHUNK
                qsl = slice(qlo, qlo + CHUNK)
                o_ps = ps_o.tile([D1, CHUNK], f32, tag="o")
                ngrp = SSLABS // GRP

                def adr():
                    if dr is not None:
                        return dr
                    # mild draining: one PE-unit per exp shadow, two only
                    # when the backlog is deep (tuned by sim scan)
                    return 2 if len(filler) > 80 else 1

                def s_group(grp):
                    s_ps = ps_s.tile([128, GRP, CHUNK], f32, tag="s", name="s_ps")
                    for kk in range(GRP):
                        k = grp * GRP + kk
                        nc.tensor.matmul(
                            s_ps[:, kk, :],
                            KT[64 * h:64 * h + 64,
                               b * S + 128 * k: b * S + 128 * (k + 1)],
                            QT[64 * h:64 * h + 64, qsl],
                            start=True, stop=True)
                    pt = pp.tile([128, GRP, CHUNK], f16, tag="pt", name="pt")
                    nc.scalar.activation(out=pt, in_=s_ps, func=Exp)
                    return pt

                def pv_group(grp, pt):
                    for kk in range(GRP):
                        k = grp * GRP + kk
                        nc.tensor.matmul(
                            o_ps, Vaug[:, b, k, h, :], pt[:, kk, :],
                            start=(k == 0), stop=(k == SSLABS - 1))

                def norm():
                    rec = sm.tile([1, CHUNK], f32r, tag="rec")
                    nc.vector.reciprocal(rec, o_ps[HEAD_DIM:D1, :])
                    b_ps = ps_mm.tile([HEAD_DIM, CHUNK], f32, tag="mm",
                                      name="bps")
                    nc.tensor.matmul(b_ps, ones1, rec, start=True, stop=True)
                    rb = sm.tile([HEAD_DIM, CHUNK], f32, tag="rb")
                    # DVE can read only one PSUM operand: stage rb in SBUF
                    if rb_act:
                        nc.scalar.copy(rb, b_ps)
                    else:
                        nc.vector.tensor_copy(rb, b_ps)
                    nc.vector.tensor_mul(
                        OT[64 * h:64 * h + 64, qsl], o_ps[0:HEAD_DIM, :], rb)

                for grp in range(ngrp):
                    pt = s_group(grp)
                    attq.append(lambda grp=grp, pt=pt: pv_group(grp, pt))
                    drain(adr())  # filler PE work runs in the exp shadow
                    while len(attq) > PVLAG:
                        attq_run()
                attq.append(norm)

            def oproj_tile_thunks(t, eng="dve"):
                # token tile t (tokens 128t..128t+127): 2 matmul halves into
                # PSUM, copy to an fp16 staging tile, one out-DMA
                box = {}

                def alloc():
                    box["ost"] = op.tile([128, HIDDEN], f16, tag="ost", name="ost")

                def half(nh):
                    acc = ps_mm.tile([128, CHUNK], f32, tag="mm", name="acc2")
                    nc.tensor.matmul(
                        acc, OT[:, 128 * t:128 * (t + 1)],
                        wo_sb[:, nh * CHUNK:(nh + 1) * CHUNK],
                        start=True, stop=True)
                    cp = nc.scalar.copy if eng == "act" else nc.vector.tensor_copy
                    cp(box["ost"][:, nh * CHUNK:(nh + 1) * CHUNK], acc)

                def flush():
                    nc.sync.dma_start(
                        out=out_d[128 * t:128 * (t + 1), :], in_=box["ost"])

                return [(0, alloc), (1, lambda: half(0)),
                        (1, lambda: half(1)), (0, flush)]

            # ---- emission: software-pipelined across reps. Rep r's b1
            # attention phase drains rep r+1's b0 qkv as filler (disjoint
            # QT/KT/VT/Vaug regions, so only WAR deps on finished readers),
            # so the reps-differenced steady state has no head/tail bubble.
            def queue_b0_qkv(xcs, head_too):
                if head_too:
                    for m in (1, 2, 0):
                        filler.extend(qkv_thunks(0, 0, m, xcs[0, 0]))
                for n in range(1, NCHUNK):
                    filler.append((4, lambda n=n: qkv_feat(0, n, 1, xcs[0, n])))
                    filler.append((4, lambda n=n: qkv_feat(0, n, 2, xcs[0, n])))
                for n in range(1, NCHUNK):
                    filler.extend(qkv_thunks(0, n, 0, xcs[0, n]))

            def queue_b1_qkv(xcs):
                for n in range(NCHUNK):
                    for m in (1, 2, 0):
                        filler.extend(qkv_thunks(1, n, m, xcs[1, n]))

            xcs_cur = {(0, 0): xc0}
            for n in range(1, NCHUNK):
                xcs_cur[0, n] = qkv_dma(0, n)
            for n in range(NCHUNK):
                xcs_cur[1, n] = qkv_dma(1, n)

            # rep 0 head emitted directly (nothing to overlap with yet)
            qkv_feat(0, 0, 1, xcs_cur[0, 0])
            qkv_feat(0, 0, 2, xcs_cur[0, 0])
            qkv_feat(0, 0, 0, xcs_cur[0, 0])
            nc.sync.dma_start(out=wo_sb, in_=woT[:])
            queue_b0_qkv(xcs_cur, head_too=False)
            queue_b1_qkv(xcs_cur)

            for _rep in range(reps):
                final = _rep == reps - 1
                for qc in range(NCHUNK):
                    first = _rep == 0 and qc == 0
                    attn_unit(0, 0, qc, dr=1 if first else None)
                    attn_unit(0, 1, qc, dr=1 if first else None)

                    def q_oproj(qc=qc):
                        for t in range(4 * qc, 4 * qc + 4):
                            filler.extend(oproj_tile_thunks(t))

                    attq.append(q_oproj)
                if not final:
                    xcs_nxt = {}
                    for n in range(NCHUNK):
                        xcs_nxt[0, n] = qkv_dma(0, n)
                for qc in range(NCHUNK):
                    last = final and qc == NCHUNK - 1
                    attn_unit(1, 0, qc, dr=1 if last else None,
                              rb_act=last)
                    attn_unit(1, 1, qc, dr=1 if last else None,
                              rb_act=last)
                    eng = "act" if last else "dve"

                    def q_oproj1(qc=qc, eng=eng):
                        for t in range(16 + 4 * qc, 16 + 4 * qc + 4):
                            filler.extend(oproj_tile_thunks(t, eng))

                    attq.append(q_oproj1)
                    if not final and qc == 0:
                        queue_b0_qkv(xcs_nxt, head_too=True)
                if not final:
                    for n in range(NCHUNK):
                        xcs_nxt[1, n] = qkv_dma(1, n)
                    queue_b1_qkv(xcs_nxt)
                    xcs_cur = xcs_nxt
                while attq:
                    attq_run()
                drain_all()

    _split_waits(nc)
    return nc


def make_in_maps(hidden_states, w_qkv, b_qkv, w_o, b_o):
    x16 = np.ascontiguousarray(
        np.asarray(hidden_states, dtype=np.float32).reshape(T, HIDDEN).T
    ).astype(np.float16)
    w_qkv = np.asarray(w_qkv, dtype=np.float32)
    b_qkv = np.asarray(b_qkv, dtype=np.float32)
    w_o = np.asarray(w_o, dtype=np.float32)

    in_maps = []
    for c in range(NCORES):
        rq = slice(c * FEAT, (c + 1) * FEAT)
        wq = w_qkv[0:QKV][rq] * SCALING
        wk = w_qkv[QKV:2 * QKV][rq]
        wv = w_qkv[2 * QKV:3 * QKV][rq]
        bqc = b_qkv[0:QKV][rq] * SCALING
        wqkvT = np.concatenate([wq, wk, wv], axis=0).T  # [1024, 384]
        wq3 = np.ascontiguousarray(
            wqkvT.reshape(KSLABS, 128, 3, FEAT).transpose(2, 1, 0, 3)
        ).astype(np.float16)
        in_maps.append({
            "xT": x16,
            "wq3": wq3,
            "bq": np.ascontiguousarray(bqc[:, None]),
            "woT": np.ascontiguousarray(w_o[:, rq].T).astype(np.float16),
        })
    return in_maps


def kernel(hidden_states, w_qkv, b_qkv, w_o, b_o):
    global LAST_RESULT
    from concourse.bass_utils import run_bass_kernel_spmd
    import os

    variant = os.environ.get("KERNEL_VARIANT", "")
    kw = dict(
        use_xbar="xbar" in variant and "noxbar" not in variant,
        use_act_bias="noactbias" not in variant,
        slab_dma="noslab" not in variant,
    )
    key = ("nc", variant)
    if key not in _CACHE:
        _CACHE[key] = _build(**kw)
    nc = _CACHE[key]

    b_qkv = np.asarray(b_qkv, dtype=np.float32)
    w_o = np.asarray(w_o, dtype=np.float32)
    b_o = np.asarray(b_o, dtype=np.float32)
    in_maps = make_in_maps(hidden_states, w_qkv, b_qkv, w_o, b_o)

    trace = bool(os.environ.get("KERNEL_TRACE"))
    res = run_bass_kernel_spmd(nc, in_maps, list(range(NCORES)), trace=trace)
    LAST_RESULT = res

    acc = np.zeros((T, HIDDEN), dtype=np.float64)
    for c in range(NCORES):
        acc += res.results[c]["out"]
    bv = b_qkv[2 * QKV:3 * QKV]
    acc += (b_o + w_o @ bv)
    out = acc.astype(np.float32).reshape(B, S, HIDDEN)
    return out
